# revision 6
# baseline (speedup 1.0000x reference)
"""CausalBank kernel v6: wide-moving W2 (amortized LDWEIGHTS) + pipelined head.

Per-core work:
  A) full embedding gather + transpose (replicated), pipelined per batch
  B) u/a matmuls + scan for ONE mode-tile (core c owns modes c*128..)
     -> hT_local; AllGather(bf16) -> featT h-part
  C) router computed locally per batch; W1 for the batch's two bs-quarters
     emitted immediately after (expert-sharded KC k-tiles), AllGather hid
  D) W2 over the core's 4000-wide vocab shard: per (quarter, vocab-block
     of 1000, bs-tile): 32 matmuls with N=1000 moving (one LDWEIGHTS per
     417ns of streaming instead of per 104ns) -> [128,1000] 2-bank PSUM.
     w2 stored DRAM-side as [4, P, KH, 1000] so block loads are 128x64KB
     contiguous descriptors.
"""

import os
import sys

for _p in ("/opt/trn_rl_repo",):
    if _p not in sys.path and os.path.isdir(_p):
        sys.path.insert(0, _p)

import numpy as np
import ml_dtypes

import concourse.bass as bass
import concourse.bacc as bacc
import concourse.mybir as mybir
import concourse.tile as tile
from concourse.bass import ts, ds
from concourse.bass_utils import run_bass_kernel_spmd
from concourse.masks import make_identity

B, S, D, M, H, E, V = 2, 1024, 512, 1024, 1024, 4, 32000
BS = B * S
F = M + D
NCORES = 8
VS = V // NCORES
P = 128
DT = D // P
MT = M // P
FT = F // P
HT = H // P
KH = E * HT            # 32
KC = KH // NCORES      # 4 k-tiles of W1 per core
QN = 4
QBS = BS // QN         # 512
NCB = S // 512         # u/a chunks per batch
VB = 1000              # W2 moving width per matmul
NVB = VS // VB         # 4 vocab blocks per core
BF = mybir.dt.bfloat16
F32 = mybir.dt.float32
AF = mybir.ActivationFunctionType
OP = mybir.AluOpType

_CACHE = {}
LAST_EXEC_NS = None


def _install_ntff_hook():
    import contextlib
    import ctypes
    import types

    if "antenv.axon_hooks" in sys.modules:
        return
    so_path = "/opt/axon/libaxon_pjrt.so"
    hook = None
    if os.path.exists(so_path):
        lib = ctypes.CDLL(so_path)
        if hasattr(lib, "axon_start_nrt_profile"):
            lib.axon_start_nrt_profile.argtypes = [
                ctypes.POINTER(ctypes.c_int64),
                ctypes.c_size_t,
            ]
            lib.axon_start_nrt_profile.restype = ctypes.c_int64
            lib.axon_stop_nrt_profile.argtypes = [ctypes.c_char_p]
            lib.axon_stop_nrt_profile.restype = ctypes.c_int64

            @contextlib.contextmanager
            def hook(output_dir, device_ids):
                import jax

                jax.devices()
                if device_ids:
                    ids = (ctypes.c_int64 * len(device_ids))(*device_ids)
                    rc = lib.axon_start_nrt_profile(ids, len(device_ids))
                else:
                    rc = lib.axon_start_nrt_profile(None, 0)
                if rc != 0:
                    raise RuntimeError(f"axon_start_nrt_profile rc={rc}")
                try:
                    yield
                finally:
                    n = lib.axon_stop_nrt_profile(str(output_dir).encode())
                    if n < 0:
                        raise RuntimeError(f"axon_stop_nrt_profile rc={n}")

    mod = types.ModuleType("antenv.axon_hooks")
    mod.get_axon_ntff_profile_hook = lambda: hook
    mod.set_axon_ntff_profile_hook = lambda h: None
    import antenv

    antenv.axon_hooks = mod
    sys.modules["antenv.axon_hooks"] = mod


def build_program(vs=VS, with_b2=False):
    nvb = vs // VB
    assert nvb * VB == vs
    nc = bacc.Bacc("TRN2", target_bir_lowering=False, debug=False)
    ALL = [list(range(NCORES))]

    tokens = nc.dram_tensor("tokens", [BS // P, P, 1], mybir.dt.int32, kind="ExternalInput")
    embed = nc.dram_tensor("embed", [V, D], F32, kind="ExternalInput")
    # per-core column slice of in_proj / gate_w (this core's mode tile)
    inproj = nc.dram_tensor("inproj", [DT, P, P], BF, kind="ExternalInput")
    gatew = nc.dram_tensor("gatew", [DT, P, P], BF, kind="ExternalInput")
    gateb = nc.dram_tensor("gateb", [P, 1], F32, kind="ExternalInput")
    routerw = nc.dram_tensor("routerw", [FT, P, E], BF, kind="ExternalInput")
    routerb = nc.dram_tensor("routerb", [E, 1], F32, kind="ExternalInput")
    gsel = nc.dram_tensor("gsel", [E, 1], F32, kind="ExternalInput")
    # this core's 4 (e,h) blocks of W1: [j, f_partition, f_tile, h_col]
    w1 = nc.dram_tensor("w1", [KC, P, FT, P], BF, kind="ExternalInput")
    b1 = nc.dram_tensor("b1", [P, KC], F32, kind="ExternalInput")
    # vocab-blocked transposed W2 shard: [vb, h_partition, k_tile, vocab_col]
    w2 = nc.dram_tensor("w2", [nvb, P, KH, VB], BF, kind="ExternalInput")
    b2 = nc.dram_tensor("b2", [E, vs], BF, kind="ExternalInput")
    out = nc.dram_tensor("out", [BS, vs], F32, kind="ExternalOutput")

    with tile.TileContext(nc) as tc:
        with (
            tc.tile_pool(name="const", bufs=1) as const,
            tc.tile_pool(name="persist", bufs=1) as persist,
            tc.tile_pool(name="dram", bufs=1, space="DRAM") as dpool,
        ):
            ident = const.tile([P, P], F32)
            make_identity(nc, ident[:])
            gateb_sb = const.tile([P, 1], F32)
            nc.sync.dma_start(gateb_sb[:], gateb[:])
            rw_sb = const.tile([P, FT, E], BF)
            nc.sync.dma_start(rw_sb[:], routerw[:].rearrange("f p e -> p f e"))
            rb_sb = const.tile([E, 1], F32)
            nc.sync.dma_start(rb_sb[:], routerb[:])
            ones44 = const.tile([E, E], F32)
            nc.any.memset(ones44[:], 1.0)
            b1_sb = const.tile([P, KC], F32)
            nc.sync.dma_start(b1_sb[:], b1[:])
            gsel_sb = const.tile([E, 1], F32)
            nc.sync.dma_start(gsel_sb[:], gsel[:])
            if with_b2:
                # b2 padded to a K=128 contraction tile (rows 0..3 = b2)
                b2_sb = const.tile([P, vs], BF)
                nc.any.memset(b2_sb[:], 0.0)
                nc.sync.dma_start(b2_sb[:E, :], b2[:])

            gatesT = persist.tile([E, BS], F32)
            if with_b2:
                gb_sb = persist.tile([P, BS], BF)   # gates padded to 128 K-rows
                nc.any.memset(gb_sb[:], 0.0)
            gdram1 = dpool.tile([1, BS], F32)       # this core's expert gate row

            h_ins = [dpool.tile([P, S], BF, name=f"h_in{b}") for b in range(B)]
            h_outs = [
                dpool.tile([NCORES, P, S], BF, addr_space="Shared", name=f"h_out{b}")
                for b in range(B)
            ]
            hid_ins = [dpool.tile([P, KC, QBS], BF, name=f"hid_in{q}") for q in range(QN)]
            hid_outs = [
                dpool.tile([NCORES, P, KC, QBS], BF, addr_space="Shared", name=f"hid_out{q}")
                for q in range(QN)
            ]

            # ---------------- upstream ----------------
            with (
                tc.tile_pool(name="upw", bufs=1) as upw,
                tc.tile_pool(name="gath", bufs=4) as gath,
                tc.tile_pool(name="mlpw", bufs=3) as mlpw,
                tc.tile_pool(name="gg", bufs=2) as gg,
            ):
                featT = upw.tile([P, FT, BS], BF)   # 6 MB
                w1_sb = upw.tile([P, KC, FT, P], BF)   # 1.5 MB, whole local W1
                nc.sync.dma_start(w1_sb[:], w1[:].rearrange("j p f c -> p j f c"))
                inproj_sb = upw.tile([P, DT, P], BF)
                nc.sync.dma_start(inproj_sb[:], inproj[:].rearrange("d p m -> p d m"))
                gatew_sb = upw.tile([P, DT, P], BF)
                nc.sync.dma_start(gatew_sb[:], gatew[:].rearrange("d p m -> p d m"))

                hT = upw.tile([P, BS], F32)
                u_t = upw.tile([P, BS], F32)
                a_t = upw.tile([P, BS], F32)
                hT_bf = upw.tile([P, BS], BF)
                gexp = upw.tile([E, BS], F32)
                rsum4 = upw.tile([E, BS], F32)
                g_row = upw.tile([1, BS], F32)

                with (
                    tc.tile_pool(name="ps_t", bufs=2, space="PSUM") as ps_t,
                    tc.tile_pool(name="ps_ua", bufs=1, space="PSUM") as ps_ua,
                ):
                    # PE warm-up: ~5us of throwaway matmuls to flip HAM early
                    wm = upw.tile([P, 512], BF)
                    nc.any.memset(wm[:], 0.5)
                    wps = ps_ua.tile([P, 512], F32, tag="psu")
                    for w in range(12):
                        nc.tensor.matmul(
                            wps[:], wm[:, 0:P], wm[:], start=(w == 0), stop=(w == 11)
                        )

                    for b in range(B):
                        bsl = ts(b, S)
                        # A) gather + transpose for this batch (replicated)
                        for i in range(b * (S // P), (b + 1) * (S // P)):
                            tok_t = gath.tile([P, 1], mybir.dt.int32, tag="tok")
                            nc.sync.dma_start(tok_t[:], tokens[i])
                            emb_t = gath.tile([P, D], F32, tag="emb")
                            nc.gpsimd.indirect_dma_start(
                                out=emb_t[:], out_offset=None, in_=embed[:],
                                in_offset=bass.IndirectOffsetOnAxis(ap=tok_t[:, :1], axis=0),
                            )
                            for d in range(DT):
                                pst = ps_t.tile([P, P], F32, tag="pst")
                                nc.tensor.transpose(pst[:], emb_t[:, ts(d, P)], ident[:])
                                nc.vector.tensor_copy(featT[:, MT + d, ts(i, P)], pst[:])
                        # B) u/a matmuls -> scan -> AG(h)
                        for cc_ in range(NCB):
                            c = b * NCB + cc_
                            psu = ps_ua.tile([P, 512], F32, tag="psu")
                            psa = ps_ua.tile([P, 512], F32, tag="psa")
                            for d in range(DT):
                                nc.tensor.matmul(
                                    psu[:], inproj_sb[:, d, :], featT[:, MT + d, ts(c, 512)],
                                    start=(d == 0), stop=(d == DT - 1),
                                )
                            for d in range(DT):
                                nc.tensor.matmul(
                                    psa[:], gatew_sb[:, d, :], featT[:, MT + d, ts(c, 512)],
                                    start=(d == 0), stop=(d == DT - 1),
                                )
                            nc.vector.tensor_copy(u_t[:, ts(c, 512)], psu[:])
                            nc.scalar.activation(
                                a_t[:, ts(c, 512)], psa[:], AF.Sigmoid,
                                bias=gateb_sb[:, 0:1], scale=1.0,
                            )
                        nc.vector.tensor_tensor_scan(
                            out=hT[:, bsl], data0=a_t[:, bsl], data1=u_t[:, bsl],
                            initial=0.0, op0=OP.mult, op1=OP.add,
                        )
                        nc.vector.tensor_copy(hT_bf[:, bsl], hT[:, bsl])
                        nc.sync.dma_start(h_ins[b][:], hT_bf[:, bsl])
                        nc.gpsimd.collective_compute(
                            "AllGather", OP.bypass, replica_groups=ALL,
                            ins=[h_ins[b][:]], outs=[h_outs[b][:]],
                        )

                # C) per batch: router + gates + W1 for its two quarters
                with (
                    tc.tile_pool(name="ps_r", bufs=2, space="PSUM") as ps_r,
                    tc.tile_pool(name="ps_h", bufs=3, space="PSUM") as ps_h,
                ):
                    for b in range(B):
                        bsl = ts(b, S)
                        # featT h-part for this batch
                        nc.sync.dma_start(
                            featT[:, 0:MT, bsl], h_outs[b][:].rearrange("r p s -> p r s")
                        )
                        # router, local over gathered featT
                        for cc_ in range(NCB):
                            c = b * NCB + cc_
                            psr = ps_r.tile([E, 512], F32, tag="psr")
                            for f in range(FT):
                                nc.tensor.matmul(
                                    psr[:], rw_sb[:, f, :], featT[:, f, ts(c, 512)],
                                    start=(f == 0), stop=(f == FT - 1),
                                )
                            nc.scalar.activation(
                                gexp[:, ts(c, 512)], psr[:], AF.Exp, bias=rb_sb[:], scale=1.0
                            )
                            pss = ps_r.tile([E, 512], F32, tag="pss")
                            nc.tensor.matmul(
                                pss[:], ones44[:], gexp[:, ts(c, 512)], start=True, stop=True
                            )
                            nc.vector.reciprocal(rsum4[:, ts(c, 512)], pss[:])
                        nc.vector.tensor_tensor(
                            out=gatesT[:, bsl], in0=gexp[:, bsl], in1=rsum4[:, bsl],
                            op=OP.mult,
                        )
                        if with_b2:
                            nc.vector.tensor_copy(gb_sb[:E, bsl], gatesT[:, bsl])
                        # select this core's expert gate row via one-hot matmul
                        for cc_ in range(NCB):
                            c = b * NCB + cc_
                            psgr = ps_r.tile([E, 512], F32, tag="pss")
                            nc.tensor.matmul(
                                psgr[0:1, :], gsel_sb[:], gatesT[:, ts(c, 512)],
                                start=True, stop=True,
                            )
                            nc.vector.tensor_copy(g_row[:, ts(c, 512)], psgr[0:1, :])
                        nc.sync.dma_start(gdram1[:, bsl], g_row[:, bsl])

                        # W1 (expert-sharded) for the two quarters of this batch
                        for q in (2 * b, 2 * b + 1):
                            qsl = ds(q * QBS, QBS)
                            g_t = gg.tile([P, QBS], F32, tag="g")
                            # all KC k-tiles of one core share one expert (e = c // 2)
                            nc.sync.dma_start(
                                g_t[:], gdram1[0:1, qsl].to_broadcast((P, QBS))
                            )
                            for j in range(KC):
                                psh = ps_h.tile([P, 512], F32, tag="psh")
                                for f in range(FT):
                                    nc.tensor.matmul(
                                        psh[:], w1_sb[:, j, f, :], featT[:, f, qsl],
                                        start=(f == 0), stop=(f == FT - 1),
                                    )
                                r_t = mlpw.tile([P, QBS], F32, tag="relu")
                                nc.scalar.activation(
                                    r_t[:], psh[:], AF.Relu, bias=b1_sb[:, j : j + 1], scale=1.0
                                )
                                r2_t = mlpw.tile([P, QBS], F32, tag="relu2")
                                nc.vector.tensor_tensor(out=r2_t[:], in0=r_t[:], in1=r_t[:], op=OP.mult)
                                hl_t = mlpw.tile([P, QBS], BF, tag="hl")
                                nc.vector.tensor_tensor(out=hl_t[:], in0=r2_t[:], in1=g_t[:], op=OP.mult)
                                nc.sync.dma_start(hid_ins[q][:, j, :], hl_t[:])
                            nc.gpsimd.collective_compute(
                                "AllGather", OP.bypass, replica_groups=ALL,
                                ins=[hid_ins[q][:]], outs=[hid_outs[q][:]],
                            )

            # ---------------- W2 (vocab-sharded, wide moving) ----------------
            with (
                tc.tile_pool(name="w2p", bufs=2) as w2p,
                tc.tile_pool(name="hidp", bufs=2) as hidp,
                tc.tile_pool(name="otp", bufs=2) as otp,
                tc.tile_pool(name="ps_o", bufs=3, space="PSUM") as ps_o,
            ):
                for q in range(QN):
                    hidT = hidp.tile([P, KH, QBS], BF, tag="hid")
                    for r in range(NCORES):
                        nc.sync.dma_start(hidT[:, ds(r * KC, KC), :], hid_outs[q][r])
                    for vb in range(nvb):
                        w2_t = w2p.tile([P, KH, VB], BF, tag="w2")
                        nc.sync.dma_start(w2_t[:], w2[vb])
                        HVB = VB // 2
                        for bt in range(QBS // P):
                            psoA = ps_o.tile([P, HVB], F32, tag="psoA")
                            psoB = ps_o.tile([P, HVB], F32, tag="psoB")
                            last = not with_b2
                            for k in range(KH):
                                # consecutive matmuls share the same stationary
                                # hid tile (LDWEIGHTS amortization if elided)
                                nc.tensor.matmul(
                                    psoA[:], hidT[:, k, ts(bt, P)], w2_t[:, k, 0:HVB],
                                    start=(k == 0),
                                    stop=(last and k == KH - 1),
                                )
                                nc.tensor.matmul(
                                    psoB[:], hidT[:, k, ts(bt, P)], w2_t[:, k, HVB:VB],
                                    start=(k == 0),
                                    stop=(last and k == KH - 1),
                                )
                            if with_b2:
                                nc.tensor.matmul(
                                    psoA[:],
                                    gb_sb[:, ds(q * QBS + bt * P, P)],
                                    b2_sb[:, ds(vb * VB, HVB)],
                                    start=False, stop=True,
                                )
                                nc.tensor.matmul(
                                    psoB[:],
                                    gb_sb[:, ds(q * QBS + bt * P, P)],
                                    b2_sb[:, ds(vb * VB + HVB, HVB)],
                                    start=False, stop=True,
                                )
                            o_t = otp.tile([P, VB], F32, tag="ot")
                            nc.vector.tensor_copy(o_t[:, 0:HVB], psoA[:])
                            nc.vector.tensor_copy(o_t[:, HVB:VB], psoB[:])
                            nc.sync.dma_start(
                                out[ds(q * QBS + bt * P, P), ds(vb * VB, VB)], o_t[:]
                            )

    nc.compile()
    return nc


def _to_bf16(x):
    return np.asarray(x, dtype=np.float32).astype(ml_dtypes.bfloat16)


def prepare_in_maps(inputs, vs=VS, ncores=NCORES):
    tokens = np.asarray(inputs["tokens"]).astype(np.int32).reshape(BS // P, P, 1)
    embed = np.ascontiguousarray(np.asarray(inputs["embed"], dtype=np.float32))
    inproj_f = np.asarray(inputs["in_proj"], dtype=np.float32)
    gatew_f = np.asarray(inputs["gate_w"], dtype=np.float32)
    gateb_f = np.asarray(inputs["gate_b"], dtype=np.float32)
    routerw_bf = _to_bf16(inputs["router_w"]).reshape(FT, P, E)
    routerb = np.asarray(inputs["router_b"], dtype=np.float32).reshape(E, 1)
    w1_bf = _to_bf16(inputs["w1"]).reshape(E, FT, P, HT, P).transpose(0, 3, 2, 1, 4)
    # -> [E, HT, P(f), FT, P(hc)]; flatten (e,h) into k
    w1_k = np.ascontiguousarray(w1_bf.reshape(KH, P, FT, P))
    b1_k = np.asarray(inputs["b1"], dtype=np.float32).reshape(E, HT, P).reshape(KH, P)
    w2_bf = _to_bf16(inputs["w2"]).reshape(E, HT, P, V).reshape(KH, P, V)
    b2_bf = _to_bf16(inputs["b2"])
    nvb = vs // VB
    shared = dict(tokens=tokens, embed=embed, routerb=routerb, routerw=routerw_bf)
    in_maps = []
    for c in range(ncores):
        m = dict(shared)
        msl = slice(c * P, (c + 1) * P)
        m["inproj"] = np.ascontiguousarray(_to_bf16(inproj_f[:, msl]).reshape(DT, P, P))
        m["gatew"] = np.ascontiguousarray(_to_bf16(gatew_f[:, msl]).reshape(DT, P, P))
        m["gateb"] = np.ascontiguousarray(gateb_f[msl].reshape(P, 1))
        onehot = np.zeros((E, 1), np.float32)
        onehot[c // 2, 0] = 1.0
        m["gsel"] = onehot
        m["w1"] = np.ascontiguousarray(w1_k[c * KC : (c + 1) * KC])
        m["b1"] = np.ascontiguousarray(b1_k[c * KC : (c + 1) * KC].T)
        # [KH, P, vs] -> [P, KH, vs] -> [P, KH, nvb, VB] -> [nvb, P, KH, VB]
        w2c = w2_bf[:, :, c * vs : (c + 1) * vs].transpose(1, 0, 2)
        m["w2"] = np.ascontiguousarray(
            w2c.reshape(P, KH, nvb, VB).transpose(2, 0, 1, 3)
        )
        m["b2"] = np.ascontiguousarray(b2_bf[:, c * vs : (c + 1) * vs])
        in_maps.append(m)
    return in_maps


def kernel(**inputs):
    global LAST_EXEC_NS
    trace = os.environ.get("BASS_TRACE", "") not in ("", "0")
    if trace:
        _install_ntff_hook()
    with_b2 = bool(np.any(np.asarray(inputs["b2"])))
    key = ("nc", with_b2)
    if key not in _CACHE:
        _CACHE[key] = build_program(with_b2=with_b2)
    nc = _CACHE[key]
    in_maps = prepare_in_maps(inputs)
    res = run_bass_kernel_spmd(nc, in_maps, list(range(NCORES)), trace=trace)
    LAST_EXEC_NS = res.exec_time_ns
    parts = [res.results[c]["out"] for c in range(NCORES)]
    full = np.concatenate(parts, axis=1).reshape(B, S, V).astype(np.float32)
    return full
